# revision 1
# baseline (speedup 1.0000x reference)
"""Trainium2 Bass kernel for nn_BPSpikingNet (3-layer LIF spiking net).

Strategy (data-parallel over batch, 8 NeuronCores, zero collectives):
  - Host pre-transposes x to feature-major layout per core (free), folds all
    scales into the weights, and converts to fp16 (safe: fp32-PSUM
    accumulation; the LIF threshold dynamics have huge margins here).
  - Per core: stream T=100 in chunks of 10 steps. Per layer, matmuls put
    output-features on partitions: z[o, t, b] accumulated over k-chunks in
    PSUM, then copied to SBUF by the scalar engine with fused per-partition
    bias.
  - The three layers' LIF scans run as ONE concatenated DVE scan over a
    [128, 544] state (8 o-chunks x 32b for L0, same for L1, 32 for L2),
    with layer L lagging layer L-1 by 2 chunks so the tensor engine always
    has a full wave of slack.
  - LIF step (v' = 0.5*v + 0.5*z, spike at >= 1, hard reset) is computed as
      u = (w * 0.5) + z2          [scalar_tensor_tensor]
      q = (u < 1) * 0.5           [tensor_scalar]  -> spike code {0.5, 0}
      w = (u < 1) * u             [scalar_tensor_tensor]
    where z2 = 0.5*z is folded into the weights, and the next layer consumes
    q via W_eff = -W, b_eff = 0.5*(b + rowsum(W))  (since s = 1 - 2q).
  - Output: q2 in [20, T, 32] per core; host maps s = 1 - 2q and gathers.
"""
import sys

import numpy as np

sys.path.insert(0, "/opt/trn_rl_repo")

import concourse.bass as bass
import concourse.tile as tile
from concourse import bacc, mybir
from concourse.bass_utils import run_bass_kernel_spmd


def _install_ntff_shim():
    """Provide antenv.axon_hooks (missing in the trimmed image) so that
    trace=True NTFF profiling works when requested via BASS_TRACE."""
    try:
        import antenv.axon_hooks  # noqa: F401
        return
    except ImportError:
        pass
    try:
        import types

        import antenv

        mod = types.ModuleType("antenv.axon_hooks")
        holder = {"h": None}
        mod.set_axon_ntff_profile_hook = lambda h: holder.__setitem__("h", h)
        mod.get_axon_ntff_profile_hook = lambda: holder["h"]
        sys.modules["antenv.axon_hooks"] = mod
        antenv.axon_hooks = mod
        try:
            from trn_agent_boot.trn_boot import _ntff_profile_via_ctypes

            h = _ntff_profile_via_ctypes("/opt/axon/libaxon_pjrt.so")
            if h is not None:
                mod.set_axon_ntff_profile_hook(h)
        except Exception:
            pass
    except Exception:
        pass


_install_ntff_shim()

F32 = mybir.dt.float32
F16 = mybir.dt.float16
AL = mybir.AluOpType
AF = mybir.ActivationFunctionType

T, B, FIN, HID, CLS = 100, 256, 700, 1024, 20
NCORES = 8
BC = B // NCORES            # 32 batch rows per core
TC = 10                     # timesteps per chunk
NCHUNK = T // TC            # 10
NWAVE = NCHUNK + 3          # L1 lags L0 by 2 waves, L2 lags L1 by 1
K0 = (FIN + 127) // 128     # 6 contraction chunks for layer 0
K1 = HID // 128             # 8 contraction chunks for layers 1/2
SEC = 544                   # 256 (L0 out) + 256 (L1 out) + 32 (L2 out)

_CACHE = {}
LAST_RESULT = None


def _active_window(w):
    """Column window [lo, hi) of the concat state active at wave w."""
    lo = 0 if w <= NCHUNK - 1 else (256 if w <= NCHUNK + 1 else 512)
    hi = 256 if w < 2 else (512 if w < 3 else SEC)
    return lo, hi


def _build():
    nc = bacc.Bacc(None, target_bir_lowering=False)
    XT = nc.declare_dram_parameter("XT", [K0, 128, T, BC], F16, isOutput=False)
    W0T = nc.declare_dram_parameter("W0T", [K0, 128, HID], F16, isOutput=False)
    W1T = nc.declare_dram_parameter("W1T", [K1, 128, HID], F16, isOutput=False)
    W2T = nc.declare_dram_parameter("W2T", [K1, 128, 128], F16, isOutput=False)
    BIAS = nc.declare_dram_parameter("BIAS", [128, 17], F32, isOutput=False)
    QOUT = nc.declare_dram_parameter("QOUT", [CLS, T, BC], F16, isOutput=True)

    with tile.TileContext(nc) as tc:
        with (
            tc.tile_pool(name="const", bufs=1) as cp,
            tc.tile_pool(name="zp", bufs=5) as zp,
            tc.tile_pool(name="sp", bufs=5) as sp,
            tc.tile_pool(name="up", bufs=4) as up,
            tc.tile_pool(name="pp", bufs=6, space=bass.MemorySpace.PSUM) as pp,
        ):
            xt = [cp.tile([128, T, BC], F16, tag=f"xt{k}", name=f"xt{k}") for k in range(K0)]
            w0 = [cp.tile([128, HID], F16, tag=f"w0_{k}", name=f"w0_{k}") for k in range(K0)]
            w1 = [cp.tile([128, HID], F16, tag=f"w1_{k}", name=f"w1_{k}") for k in range(K1)]
            w2 = [cp.tile([128, 128], F16, tag=f"w2_{k}", name=f"w2_{k}") for k in range(K1)]
            bias = cp.tile([128, 17], F32, tag="bias")
            wst = cp.tile([128, SEC], F16, tag="wst")
            outq = cp.tile([CLS, T, BC], F16, tag="outq")

            nc.scalar.dma_start(bias[:], BIAS[:])
            for k in range(K0):
                nc.sync.dma_start(w0[k][:], W0T[k])
                nc.sync.dma_start(xt[k][:, 0:TC, :], XT[k][:, 0:TC, :])
            for k in range(K0):
                nc.sync.dma_start(xt[k][:, TC:T, :], XT[k][:, TC:T, :])
            for k in range(K1):
                nc.sync.dma_start(w1[k][:], W1T[k])
                nc.sync.dma_start(w2[k][:], W2T[k])
            nc.gpsimd.memset(wst[:], 0.0)
            # warm the ACT activation-table during the DMA head so the first
            # psum->sbuf copy doesn't pay the ~2.7us table load
            warm = cp.tile([128, 1], F32, tag="warm")
            nc.vector.memset(warm[:], 0.0)
            nc.scalar.activation(warm[:], warm[:], AF.Identity,
                                 bias=bias[:, 0:1], scale=1.0)

            prev_s = [None, None]  # S(w-1), S(w-2)

            for w in range(NWAVE):
                zw = zp.tile([128, TC, SEC], F16, tag="z", name=f"z{w}")

                # --- fill Z(w): tensor engine + scalar-engine copies ---
                if w <= NCHUNK - 1:  # L0 chunk w
                    halves = 2 if w == 0 else 1
                    half = TC // halves
                    for hv in range(halves):
                        tsl = slice(hv * half, (hv + 1) * half)
                        for o in range(8):
                            ps = pp.tile([128, half, BC], F32, tag="ps", name="ps")
                            for k in range(K0):
                                nc.tensor.matmul(
                                    ps[:],
                                    w0[k][:, o * 128:(o + 1) * 128],
                                    xt[k][:, w * TC + hv * half:w * TC + (hv + 1) * half, :],
                                    start=(k == 0), stop=(k == K0 - 1),
                                )
                            nc.scalar.activation(
                                zw[:, tsl, o * 32:(o + 1) * 32], ps[:],
                                AF.Identity, bias=bias[:, o:o + 1], scale=1.0,
                            )
                if 2 <= w <= NCHUNK + 1:  # L1 chunk w-2, consumes S(w-2) L0 cols
                    s_in = prev_s[1]
                    halves = 2 if w == NCHUNK + 1 else 1
                    half = TC // halves
                    for hv in range(halves):
                        tsl = slice(hv * half, (hv + 1) * half)
                        for o in range(8):
                            ps = pp.tile([128, half, BC], F32, tag="ps", name="ps")
                            for k in range(K1):
                                nc.tensor.matmul(
                                    ps[:],
                                    w1[k][:, o * 128:(o + 1) * 128],
                                    s_in[:, tsl, k * 32:(k + 1) * 32],
                                    start=(k == 0), stop=(k == K1 - 1),
                                )
                            nc.scalar.activation(
                                zw[:, tsl, 256 + o * 32:256 + (o + 1) * 32], ps[:],
                                AF.Identity, bias=bias[:, 8 + o:9 + o], scale=1.0,
                            )
                if 3 <= w <= NCHUNK + 2:  # L2 chunk w-3, consumes S(w-1) L1 cols
                    s_in = prev_s[0]
                    nparts = 2
                    half = TC // nparts
                    for hv in range(nparts):
                        tsl = slice(hv * half, (hv + 1) * half)
                        ps = pp.tile([128, half, BC], F32, tag="ps2", name="ps2", bufs=2)
                        for k in range(K1):
                            nc.tensor.matmul(
                                ps[:],
                                w2[k][:],
                                s_in[:, tsl, 256 + k * 32:256 + (k + 1) * 32],
                                start=(k == 0), stop=(k == K1 - 1),
                            )
                        nc.scalar.activation(
                            zw[:, tsl, 512:SEC], ps[:, :, 0:32],
                            AF.Identity, bias=bias[:, 16:17], scale=1.0,
                        )

                # --- scan wave w: 10 LIF steps over the active window ---
                lo, hi = _active_window(w)
                sw = sp.tile([128, TC, SEC], F16, tag="s", name=f"s{w}")
                for t in range(TC):
                    # state wst = 0.5 * v_post; u = v_pre; r = q = 0.5*(u<1)
                    u = up.tile([128, SEC], F16, tag="u", name="u")
                    nc.vector.tensor_tensor(
                        u[:, lo:hi], wst[:, lo:hi], zw[:, t, lo:hi], op=AL.add,
                    )
                    nc.vector.tensor_scalar(
                        sw[:, t, lo:hi], u[:, lo:hi], 1.0, 0.5,
                        op0=AL.is_lt, op1=AL.mult,
                    )
                    nc.vector.tensor_tensor(
                        wst[:, lo:hi], u[:, lo:hi], sw[:, t, lo:hi], op=AL.mult,
                    )

                if w >= 3:  # collect L2 spikes (chunk w-3)
                    nc.scalar.copy(
                        outq[:, (w - 3) * TC:(w - 2) * TC, :],
                        sw[0:CLS, :, 512:SEC],
                    )

                prev_s = [sw, prev_s[0]]

            nc.sync.dma_start(QOUT[:], outq[:])

    nc.compile()
    return nc


def _get_nc():
    if "nc" not in _CACHE:
        _CACHE["nc"] = _build()
    return _CACHE["nc"]


def _get_runner():
    """Build (once) a cached jitted SPMD executable over the 8 cores.

    Mirrors concourse.bass2jax.run_bass_via_pjrt's multi-core branch but
    keeps the jitted function alive so repeated kernel() calls don't
    re-trace/re-compile.
    """
    if "runner" in _CACHE:
        return _CACHE["runner"]
    import jax
    from jax.sharding import Mesh, PartitionSpec
    from jax.experimental.shard_map import shard_map
    from concourse import bass2jax

    nc = _get_nc()
    bass2jax.install_neuronx_cc_hook()
    partition_name = (
        nc.partition_id_tensor.name if nc.partition_id_tensor else None
    )
    in_names, out_names, out_avals, zero_shapes = [], [], [], []
    for alloc in nc.m.functions[0].allocations:
        if not isinstance(alloc, mybir.MemoryLocationSet):
            continue
        name = alloc.memorylocations[0].name
        if alloc.kind == "ExternalInput":
            if name != partition_name:
                in_names.append(name)
        elif alloc.kind == "ExternalOutput":
            shape = tuple(alloc.tensor_shape)
            dtype = mybir.dt.np(alloc.dtype)
            out_names.append(name)
            out_avals.append(jax.core.ShapedArray(shape, dtype))
            zero_shapes.append((shape, dtype))
    n_params = len(in_names)
    all_in = in_names + out_names
    if partition_name is not None:
        all_in = all_in + [partition_name]

    def _body(*args):
        operands = list(args)
        if partition_name is not None:
            operands.append(bass2jax.partition_id_tensor())
        outs = bass2jax._bass_exec_p.bind(
            *operands,
            out_avals=tuple(out_avals),
            in_names=tuple(all_in),
            out_names=tuple(out_names),
            lowering_input_output_aliases=(),
            sim_require_finite=True,
            sim_require_nnan=True,
            nc=nc,
        )
        return tuple(outs)

    devices = jax.devices()[:NCORES]
    mesh = Mesh(np.asarray(devices), ("core",))
    donate = tuple(range(n_params, n_params + len(out_names)))
    sharded = jax.jit(
        shard_map(
            _body, mesh=mesh,
            in_specs=(PartitionSpec("core"),) * (n_params + len(out_names)),
            out_specs=(PartitionSpec("core"),) * len(out_names),
            check_rep=False,
        ),
        donate_argnums=donate, keep_unused=True,
    )

    def run(in_maps):
        concat_in = [
            np.concatenate([np.asarray(m[nm]) for m in in_maps], axis=0)
            for nm in in_names
        ]
        concat_zeros = [
            np.zeros((NCORES * sh[0], *sh[1:]), dt) for sh, dt in zero_shapes
        ]
        out_arrs = sharded(*concat_in, *concat_zeros)
        return [
            {
                nm: np.asarray(out_arrs[i]).reshape(NCORES, *out_avals[i].shape)[c]
                for i, nm in enumerate(out_names)
            }
            for c in range(NCORES)
        ]

    _CACHE["runner"] = run
    return run


def kernel(x_tbf, W0, b0, W1, b1, W2, b2):
    global LAST_RESULT
    import os

    x = np.asarray(x_tbf, np.float32)
    W0 = np.asarray(W0, np.float32)
    W1 = np.asarray(W1, np.float32)
    W2 = np.asarray(W2, np.float32)
    b0 = np.asarray(b0, np.float32)
    b1 = np.asarray(b1, np.float32)
    b2 = np.asarray(b2, np.float32)

    # weights: fold the 0.5 (leak) scale and the q-code correction (s = 1-2q)
    w0t = np.zeros((K0 * 128, HID), np.float16)
    w0t[:FIN] = (0.5 * W0.T).astype(np.float16)
    w1t = (-W1.T).astype(np.float16)                      # [1024, 1024]
    w2t = np.zeros((HID, 128), np.float16)
    w2t[:, :CLS] = (-W2.T).astype(np.float16)

    bias_arr = np.zeros((128, 17), np.float32)
    bias_arr[:, 0:8] = (0.5 * b0).reshape(8, 128).T
    b1e = 0.5 * (b1.astype(np.float64) + W1.astype(np.float64).sum(axis=1))
    bias_arr[:, 8:16] = b1e.astype(np.float32).reshape(8, 128).T
    b2e = 0.5 * (b2.astype(np.float64) + W2.astype(np.float64).sum(axis=1))
    bias_arr[:CLS, 16] = b2e.astype(np.float32)

    w0t_r = w0t.reshape(K0, 128, HID)
    w1t_r = w1t.reshape(K1, 128, HID)
    w2t_r = w2t.reshape(K1, 128, 128)

    in_maps = []
    for c in range(NCORES):
        xs = x[:, c * BC:(c + 1) * BC, :]                 # [T, BC, FIN]
        xt = np.zeros((K0 * 128, T, BC), np.float16)
        xt[:FIN] = xs.transpose(2, 0, 1).astype(np.float16)
        in_maps.append({
            "XT": np.ascontiguousarray(xt.reshape(K0, 128, T, BC)),
            "W0T": w0t_r, "W1T": w1t_r, "W2T": w2t_r, "BIAS": bias_arr,
        })

    if os.environ.get("BASS_TRACE"):
        nc = _get_nc()
        LAST_RESULT = run_bass_kernel_spmd(
            nc, in_maps, list(range(NCORES)),
            trace=True,
            tmpdir=os.environ.get("BASS_TRACE_DIR"),
        )
        results = LAST_RESULT.results
    else:
        results = _get_runner()(in_maps)

    out = np.empty((T, B, CLS), np.float32)
    for c in range(NCORES):
        q = results[c]["QOUT"].astype(np.float32)  # [CLS, T, BC]
        out[:, c * BC:(c + 1) * BC, :] = (1.0 - 2.0 * q).transpose(1, 2, 0)
    return out

